# revision 2
# baseline (speedup 1.0000x reference)
"""AttentionAugmentation2D kernel for 8 Trainium2 NeuronCores — v3.

Data-parallel over batch (B=8 -> 1 batch element per core).

Math (per batch, per head; H=W=32, L=H*W=1024, dh=32):
  logits[(x,y),(x',y')] = q.k + q.krw[y'-y+31] + q.krh[x'-x+31]
Both relative terms are folded into a single K=96 matmul:
  Q_aug = [qT; skew_w(q @ krw^T); skew_h(q @ krh^T)]   (96 x 1024 per head)
  K_aug = [kT; onehot32(y'); onehot32(x')]             (96 x 1024 per head)
logits are computed transposed (keys on partitions) so that exp(logitsT)
is directly the rhs of the attention@V matmul.

v3 structure:
  - the whole input ships as one bf16 [L, 768] host tensor; q/k are
    transposed by the DMA XBAR directly DRAM -> SBUF (zero engine/PE
    time) and V is DMA'd straight into its [128, t, h, d] layout.
    The XBAR stream writes [d, pos*4 + headblock]; the aug tensors use
    that interleaved column layout and all matmuls read stride-4
    slices. After the QK matmul the interleave vanishes (output order
    follows the rhs stream), so exp/attention@V/finish see pos order.
  - bf16 matmul inputs run at 1 cycle/row at ANY free size, so the rel
    matmuls are split per 4-head half: half 0 runs as soon as the first
    q transpose lands, half 1 is interleaved into attention phase 0
    (its PSUM tiles ride the lt pool ring); heads 4-7 only need it
    ~4 phases later.
  - exp runs on Pool (qc0) and ACT (qc1) via raw InstActivation with
    the 1/sqrt(dh) scale folded into the activation scale operand.
  - attention@V accumulates into per-head [33,L] PSUM tiles from a
    2-deep pool; per-head finish = 2 evac halves (DVE), 8 PE transposes
    into one PSUM tile, strided reciprocal, broadcast multiply.
"""

import numpy as np
import ml_dtypes

import concourse.bass as bass
import concourse.mybir as mybir
import concourse.tile as tile
from concourse import bacc
from concourse.bass_utils import run_bass_kernel_spmd

FP = mybir.dt.float32
FPR = mybir.dt.float32r
BF = mybir.dt.bfloat16
AF = mybir.ActivationFunctionType

B = 8
H = W = 32
NH = 8
DH = 32          # per-head depth for q/k/v
L = H * W        # 1024 positions
SCALE = float(DH) ** -0.5
NT = L // 128    # 8 position tiles


def _build_onehot():
    # rows 0-31: onehot of y' = key % 32 ; rows 32-63: onehot of x' = key//32
    # pre-interleaved to the XBAR column layout col = pos*4 + hb, bf16 exact
    oh = np.zeros((64, L), dtype=np.float32)
    k = np.arange(L)
    oh[k % 32, k] = 1.0
    oh[32 + k // 32, k] = 1.0
    ohi = np.repeat(oh[:, :, None], 4, axis=2).reshape(64, 4 * L)
    return np.ascontiguousarray(ohi.astype(ml_dtypes.bfloat16))


def _build_nc():
    nc = bacc.Bacc(
        "TRN2",
        target_bir_lowering=False,
        debug=False,
        enable_asserts=True,
        num_devices=B,
    )
    xbf = nc.declare_dram_parameter("xbf", [L, 3 * NH * DH], BF, isOutput=False)
    qki = nc.declare_dram_parameter("qki", [128, 4 * L], BF, isOutput=False)
    krwh = nc.declare_dram_parameter("krwhT", [DH, 2 * (2 * W - 1)], BF, isOutput=False)
    oneh = nc.declare_dram_parameter("oneh", [64, 4 * L], BF, isOutput=False)
    identb = nc.declare_dram_parameter("identb", [128, 128], BF, isOutput=False)
    out = nc.declare_dram_parameter("out", [L, NH * DH], FP, isOutput=True)

    def mkexp(eng, out_ap, in_ap, tag):
        # raw InstActivation so exp can run on Pool too; scale folds the
        # 1/sqrt(dh) so all upstream copies stay pure
        eng.add_instruction(
            mybir.InstActivation(
                name=f"vexp_{tag}",
                func=AF.Exp,
                ins=[
                    eng.lower_ap(in_ap),
                    mybir.ImmediateValue(dtype=FP, value=0.0),
                    mybir.ImmediateValue(dtype=FP, value=SCALE),
                    mybir.ImmediateValue(dtype=FP, value=0.0),
                ],
                outs=[eng.lower_ap(out_ap)],
            )
        )

    def copy_on(eng, dst, src):
        if eng is nc.scalar:
            eng.copy(dst, src)
        else:
            eng.tensor_copy(dst, src)

    with tile.TileContext(nc) as tc:
        with (
            tc.tile_pool(name="const", bufs=1) as cp,
        ):
            ident = cp.tile([128, 128], BF)
            krwh_sb = cp.tile([DH, 2 * (2 * W - 1)], BF)
            krw_sb = krwh_sb[:, 0 : 2 * W - 1]
            krh_sb = krwh_sb[:, 2 * W - 1 :]

            # interleaved column layout: col(half, pos, hb) =
            #   half*4096 + pos*4 + hb,  head h = half*4 + hb
            QaugT = cp.tile([96, 2, L, 4], BF)
            KaugT = cp.tile([96, 2, L, 4], BF)
            Vaug = cp.tile([128, NT, NH, DH + 2], BF)

            # The DMA transfers serialize on one pipe, and the scheduler
            # round-robins SWDGE/HWDGE with ~2us link latency on each
            # switch — so keep the whole chain on HWDGE, in deadline
            # order: half-0 data first, then V, half-1, and the SWDGE
            # ident (needed ~20us in) dead last.
            # q/k arrive host-pre-transposed+interleaved, packed as four
            # 32-row groups across 128 partitions: one fast DMA, then four
            # partition-shift copies (DVE 4x for half-0, Pool for half-1)
            qkst = cp.tile([128, 4 * L], BF, name="qkst")
            with tc.high_priority():
                nc.sync.dma_start(out=qkst, in_=qki[:])
                nc.sync.dma_start(out=krwh_sb, in_=krwh[:])
                nc.sync.dma_start(
                    out=KaugT[32:96, 0].rearrange("p f h -> p (f h)"), in_=oneh[:]
                )
                nc.sync.dma_start(out=ident, in_=identb[:])
            # V straight into its SBUF layout (leaves the ones column gap);
            # per-t pieces keep the DMA APs within 3 dims
            xvr = xbf.rearrange("(t p) c -> p t c", p=128)
            with tc.tile_wait_until(0.006):
                for t in range(NT):
                    nc.sync.dma_start(
                        out=Vaug[:, t, :, 0:DH],
                        in_=xvr[:, t, 512:768].rearrange("p (h d) -> p h d", d=DH),
                    )
            with tc.tile_wait_until(0.010):
                nc.sync.dma_start(
                    out=KaugT[32:96, 1].rearrange("p f h -> p (f h)"), in_=oneh[:]
                )
            # ones column for the softmax denominator: engine memset, no DMA
            nc.vector.memset(
                Vaug[:, :, :, DH : DH + 1].rearrange("p t h o -> p (t h o)"), 1.0
            )

            # rows: 0:32 q half0, 32:64 q half1, 64:96 k half0, 96:128 k half1
            nc.vector.tensor_copy(
                QaugT[0:32, 0].rearrange("p f h -> p (f h)"), qkst[0:32, :]
            )
            nc.vector.tensor_copy(
                KaugT[0:32, 0].rearrange("p f h -> p (f h)"), qkst[64:96, :]
            )
            nc.gpsimd.tensor_copy(
                QaugT[0:32, 1].rearrange("p f h -> p (f h)"), qkst[32:64, :]
            )
            nc.gpsimd.tensor_copy(
                KaugT[0:32, 1].rearrange("p f h -> p (f h)"), qkst[96:128, :]
            )

            out_sb = cp.tile([128, NT, NH * DH], FP)

            # rel views (interleaved): free ordering per mm is (hb, x|y)
            q_i = QaugT[0:32]                                  # [32,2,L,4]
            qr = q_i.rearrange("p a (x y) h -> p a h x y", y=W)
            wd = QaugT[32:64].rearrange("p a (x y) h -> p a h x y", y=W)
            hd = QaugT[64:96].rearrange("p a (x y) h -> p a h x y", y=W)

            def rel_group(pool, half, g, wdir, evac_engs, tag="rp"):
                # one y(or x)-group of 4 pre-skewed rel matmuls for one
                # 4-head half, then 2 evac halves
                rp = pool.tile([32, 4, 4, 32], FP, tag=tag, name=f"rp{half}_{wdir}_{g}")
                for i in range(4):
                    v = 4 * g + i
                    if wdir:
                        nc.tensor.matmul(
                            rp[:, i],
                            lhsT=krw_sb[:, 31 - v : 63 - v],
                            rhs=qr[:, half, :, :, v],
                            start=True,
                            stop=True,
                        )
                    else:
                        nc.tensor.matmul(
                            rp[:, i],
                            lhsT=krh_sb[:, 31 - v : 63 - v],
                            rhs=qr[:, half, :, v, :],
                            start=True,
                            stop=True,
                        )
                if wdir:
                    dst = wd[:, half, :, :, 4 * g : 4 * g + 4].rearrange(
                        "p h x i -> p i h x"
                    )
                else:
                    dst = hd[:, half, :, 4 * g : 4 * g + 4, :].rearrange(
                        "p h i y -> p i h y"
                    )
                copy_on(evac_engs[0], dst[:, 0:2], rp[:, 0:2])
                copy_on(evac_engs[1], dst[:, 2:4], rp[:, 2:4])

            # ---------------- rel half 0 (heads 0-3) ----------------------
            rel0_rot = [(nc.vector, nc.scalar), (nc.scalar, nc.vector),
                        (nc.vector, nc.scalar), (nc.scalar, nc.vector)]
            with tc.tile_pool(name="ps_rel", bufs=6, space="PSUM") as ps_rel:
                for g in range(8):
                    rel_group(ps_rel, 0, g, True, rel0_rot[g % 4])
                for g in range(8):
                    rel_group(ps_rel, 0, g, False, rel0_rot[(g + 1) % 4])

            # ---------------- attention over heads ------------------------
            with (
                tc.tile_pool(name="wt", bufs=2) as wtp,
                tc.tile_pool(name="at", bufs=2) as atp,
                tc.tile_pool(name="sm", bufs=4) as smp,
                tc.tile_pool(name="ps_lt", bufs=2, space="PSUM") as ps_lt,
                tc.tile_pool(name="ps_av", bufs=2, space="PSUM") as ps_av,
            ):
                wts = {}
                avs = {}
                at_sbs = {}

                def evac_head(h, engs):
                    av = avs.pop(h)
                    at_sb = atp.tile([DH + 1, L], BF, tag="at", name=f"at{h}")
                    at_sbs[h] = at_sb
                    copy_on(engs[0], at_sb[:, 0:512], av[:, 0:512])
                    copy_on(engs[1], at_sb[:, 512:1024], av[:, 512:1024])

                def finish_ft(h, t0, t1, tt_eng):
                    at_sb = at_sbs[h]
                    ftile = ps_lt.tile(
                        [128, t1 - t0, DH + 2], BF, tag="lt", name=f"ft{h}_{t0}"
                    )
                    for t in range(t0, t1):
                        nc.tensor.transpose(
                            ftile[:, t - t0, 0 : DH + 1],
                            at_sb[:, t * 128 : (t + 1) * 128],
                            ident[0 : DH + 1, 0 : DH + 1],
                        )
                    rcp = smp.tile([128, t1 - t0], FP, tag="rcp")
                    nc.vector.reciprocal(rcp, ftile[:, :, DH])
                    rcp_b = bass.AP(
                        tensor=rcp.tensor,
                        offset=rcp.offset,
                        ap=[rcp.ap[0], rcp.ap[1], [0, DH]],
                    )
                    tt_eng.tensor_tensor(
                        out_sb[:, t0:t1, h * DH : (h + 1) * DH],
                        ftile[:, :, 0:DH],
                        rcp_b,
                        mybir.AluOpType.mult,
                    )

                # rel half 1 groups spread over phases 0-3 (heads 4-7 only
                # need them from phase 4); evacs on DVE (Pool cannot access
                # PSUM on real hardware)
                rel1 = [(g, True) for g in range(8)] + [(g, False) for g in range(8)]
                rel1_rot = [(nc.vector, nc.vector), (nc.vector, nc.vector)]

                for h in range(NH + 1):
                    if h < NH:
                        wts[h] = wtp.tile(
                            [128, NT, L], BF, tag="wt", name=f"wt{h}"
                        )
                        ha, hb = h // 4, h % 4
                    if h > 0:
                        avp = ps_av.tile([DH + 1, L], FP, tag="av", name=f"av{h-1}")
                        avs[h - 1] = avp
                        WTp = wts[h - 1]
                    for kt in range(NT):
                        if kt == 2 and h >= 2:
                            finish_ft(h - 2, 0, NT, nc.vector)
                            del at_sbs[h - 2]
                        if h < NH:
                            lt = ps_lt.tile([128, L], FP, tag="lt")
                            for qc in range(2):
                                nc.tensor.matmul(
                                    lt[:, qc * 512 : (qc + 1) * 512],
                                    lhsT=KaugT[:, ha, kt * 128 : (kt + 1) * 128, hb],
                                    rhs=QaugT[:, ha, qc * 512 : (qc + 1) * 512, hb],
                                    start=True,
                                    stop=True,
                                )
                            # exp may only run on the Activation engine
                            nc.scalar.activation(
                                wts[h][:, kt, :], lt, AF.Exp, scale=SCALE
                            )
                        if h < 4 and kt % 2 == 0:
                            g, wdir = rel1[4 * h + kt // 2]
                            rel_group(ps_av, 1, g, wdir, rel1_rot[kt % 2], tag="av")
                        if h > 0:
                            for qc in range(2):
                                nc.tensor.matmul(
                                    avp[:, qc * 512 : (qc + 1) * 512],
                                    lhsT=Vaug[:, kt, h - 1, 0 : DH + 1],
                                    rhs=WTp[:, kt, qc * 512 : (qc + 1) * 512],
                                    start=(kt == 0),
                                    stop=(kt == NT - 1),
                                )
                    if h > 0:
                        del wts[h - 1]
                        if h == NH:
                            evac_head(h - 1, (nc.vector, nc.scalar))
                        else:
                            evac_head(h - 1, (nc.vector, nc.vector))

                # tail: last head's finish interleaved with the out stores
                out_r = out.rearrange("(t p) c -> p t c", p=128)
                finish_ft(NH - 1, 0, NT // 2, nc.vector)
                for t in range(NT // 2):
                    eng = (nc.sync, nc.scalar)[t % 2]
                    eng.dma_start(out=out_r[:, t, :], in_=out_sb[:, t, :])
                finish_ft(NH - 1, NT // 2, NT, nc.vector)
                for t in range(NT // 2, NT):
                    eng = (nc.sync, nc.scalar)[t % 2]
                    eng.dma_start(out=out_r[:, t, :], in_=out_sb[:, t, :])
    nc.compile()
    return nc


_NC_CACHE = None


def kernel(inputs: np.ndarray, key_rel_w: np.ndarray, key_rel_h: np.ndarray) -> np.ndarray:
    global _NC_CACHE
    xf32 = inputs.astype(np.float32).reshape(B, L, 3 * NH * DH)
    xbf = np.ascontiguousarray(xf32.astype(ml_dtypes.bfloat16))
    # [g, hb, d, pos] -> [g*32+d, pos*4+hb]
    qki = np.ascontiguousarray(
        xf32[:, :, 0:512].transpose(0, 2, 1).reshape(B, 4, 4, 32, L)
        .transpose(0, 1, 3, 4, 2).reshape(B, 128, 4 * L)
        .astype(ml_dtypes.bfloat16)
    )
    krwhT = np.ascontiguousarray(
        np.concatenate([key_rel_w, key_rel_h], axis=0)
        .astype(np.float32).T.astype(ml_dtypes.bfloat16)
    )
    oneh = _build_onehot()

    if _NC_CACHE is None:
        _NC_CACHE = _build_nc()
    nc = _NC_CACHE

    identb = np.eye(128, dtype=np.float32).astype(ml_dtypes.bfloat16)
    in_maps = [
        {"xbf": xbf[b], "qki": qki[b], "krwhT": krwhT, "oneh": oneh,
         "identb": identb}
        for b in range(B)
    ]
    res = run_bass_kernel_spmd(nc, in_maps, list(range(B)))
    o = np.stack([res.results[b]["out"] for b in range(B)], axis=0)
    return np.ascontiguousarray(o.reshape(B, H, W, NH * DH).astype(np.float32))


# revision 8
# speedup vs baseline: 1.0215x; 1.0215x over previous
"""AttentionAugmentation2D kernel for 8 Trainium2 NeuronCores — v3.

Data-parallel over batch (B=8 -> 1 batch element per core).

Math (per batch, per head; H=W=32, L=H*W=1024, dh=32):
  logits[(x,y),(x',y')] = q.k + q.krw[y'-y+31] + q.krh[x'-x+31]
Both relative terms are folded into a single K=96 matmul:
  Q_aug = [qT; skew_w(q @ krw^T); skew_h(q @ krh^T)]   (96 x 1024 per head)
  K_aug = [kT; onehot32(y'); onehot32(x')]             (96 x 1024 per head)
logits are computed transposed (keys on partitions) so that exp(logitsT)
is directly the rhs of the attention@V matmul.

v3 structure:
  - the whole input ships as one bf16 [L, 768] host tensor; q/k are
    transposed by the DMA XBAR directly DRAM -> SBUF (zero engine/PE
    time) and V is DMA'd straight into its [128, t, h, d] layout.
    The XBAR stream writes [d, pos*4 + headblock]; the aug tensors use
    that interleaved column layout and all matmuls read stride-4
    slices. After the QK matmul the interleave vanishes (output order
    follows the rhs stream), so exp/attention@V/finish see pos order.
  - bf16 matmul inputs run at 1 cycle/row at ANY free size, so the rel
    matmuls are split per 4-head half: half 0 runs as soon as the first
    q transpose lands, half 1 is interleaved into attention phase 0
    (its PSUM tiles ride the lt pool ring); heads 4-7 only need it
    ~4 phases later.
  - exp runs on Pool (qc0) and ACT (qc1) via raw InstActivation with
    the 1/sqrt(dh) scale folded into the activation scale operand.
  - attention@V accumulates into per-head [33,L] PSUM tiles from a
    2-deep pool; per-head finish = 2 evac halves (DVE), 8 PE transposes
    into one PSUM tile, strided reciprocal, broadcast multiply.
"""

import numpy as np
import ml_dtypes

import concourse.bass as bass
import concourse.mybir as mybir
import concourse.tile as tile
from concourse import bacc
from concourse.bass_utils import run_bass_kernel_spmd

FP = mybir.dt.float32
FPR = mybir.dt.float32r
BF = mybir.dt.bfloat16
AF = mybir.ActivationFunctionType

B = 8
H = W = 32
NH = 8
DH = 32          # per-head depth for q/k/v
L = H * W        # 1024 positions
SCALE = float(DH) ** -0.5
NT = L // 128    # 8 position tiles


def _build_onehot():
    # rows 0-31: onehot of y' = key % 32 ; rows 32-63: onehot of x' = key//32
    # pre-interleaved to the XBAR column layout col = pos*4 + hb, bf16 exact
    oh = np.zeros((64, L), dtype=np.float32)
    k = np.arange(L)
    oh[k % 32, k] = 1.0
    oh[32 + k // 32, k] = 1.0
    ohi = np.repeat(oh[:, :, None], 4, axis=2).reshape(64, 4 * L)
    return np.ascontiguousarray(ohi.astype(ml_dtypes.bfloat16))


def _build_nc():
    nc = bacc.Bacc(
        "TRN2",
        target_bir_lowering=False,
        debug=False,
        enable_asserts=True,
        num_devices=B,
    )
    xbf = nc.declare_dram_parameter("xbf", [L, 3 * NH * DH], BF, isOutput=False)
    qki = nc.declare_dram_parameter("qki", [128, 4 * L], BF, isOutput=False)
    krwh = nc.declare_dram_parameter("krwhT", [DH, 2 * (2 * W - 1)], BF, isOutput=False)
    oneh = nc.declare_dram_parameter("oneh", [64, 4 * L], BF, isOutput=False)
    identb = nc.declare_dram_parameter("identb", [128, 128], BF, isOutput=False)
    out = nc.declare_dram_parameter("out", [L, NH * DH], FP, isOutput=True)

    def mkexp(eng, out_ap, in_ap, tag):
        # raw InstActivation so exp can run on Pool too; scale folds the
        # 1/sqrt(dh) so all upstream copies stay pure
        eng.add_instruction(
            mybir.InstActivation(
                name=f"vexp_{tag}",
                func=AF.Exp,
                ins=[
                    eng.lower_ap(in_ap),
                    mybir.ImmediateValue(dtype=FP, value=0.0),
                    mybir.ImmediateValue(dtype=FP, value=SCALE),
                    mybir.ImmediateValue(dtype=FP, value=0.0),
                ],
                outs=[eng.lower_ap(out_ap)],
            )
        )

    def copy_on(eng, dst, src):
        if eng is nc.scalar:
            eng.copy(dst, src)
        else:
            eng.tensor_copy(dst, src)

    with tile.TileContext(nc) as tc:
        with (
            tc.tile_pool(name="const", bufs=1) as cp,
        ):
            ident = cp.tile([128, 128], BF)
            krwh_sb = cp.tile([DH, 2 * (2 * W - 1)], BF)
            krw_sb = krwh_sb[:, 0 : 2 * W - 1]
            krh_sb = krwh_sb[:, 2 * W - 1 :]

            # interleaved column layout: col(half, pos, hb) =
            #   half*4096 + pos*4 + hb,  head h = half*4 + hb
            QaugT = cp.tile([96, 2, L, 4], BF)
            KaugT = cp.tile([96, 2, L, 4], BF)
            Vaug = cp.tile([128, NT, NH, DH + 2], BF)

            # The DMA transfers serialize on one pipe, and the scheduler
            # round-robins SWDGE/HWDGE with ~2us link latency on each
            # switch — so keep the whole chain on HWDGE, in deadline
            # order: half-0 data first, then V, half-1, and the SWDGE
            # ident (needed ~20us in) dead last.
            # q/k arrive host-pre-transposed+interleaved, packed as four
            # 32-row groups across 128 partitions: one fast DMA, then four
            # partition-shift copies (DVE 4x for half-0, Pool for half-1)
            qkst = cp.tile([128, 4 * L], BF, name="qkst")
            with tc.high_priority():
                nc.sync.dma_start(out=qkst, in_=qki[:])
                nc.sync.dma_start(out=krwh_sb, in_=krwh[:])
                nc.sync.dma_start(
                    out=KaugT[32:96, 0].rearrange("p f h -> p (f h)"), in_=oneh[:]
                )
                nc.sync.dma_start(out=ident, in_=identb[:])
            # V straight into its SBUF layout (leaves the ones column gap);
            # per-t pieces keep the DMA APs within 3 dims
            xvr = xbf.rearrange("(t p) c -> p t c", p=128)
            with tc.tile_wait_until(0.006):
                for t in range(NT):
                    nc.sync.dma_start(
                        out=Vaug[:, t, :, 0:DH],
                        in_=xvr[:, t, 512:768].rearrange("p (h d) -> p h d", d=DH),
                    )
            with tc.tile_wait_until(0.010):
                nc.sync.dma_start(
                    out=KaugT[32:96, 1].rearrange("p f h -> p (f h)"), in_=oneh[:]
                )
            # ones column for the softmax denominator: engine memset, no DMA
            nc.vector.memset(
                Vaug[:, :, :, DH : DH + 1].rearrange("p t h o -> p (t h o)"), 1.0
            )

            # rows: 0:32 q half0, 32:64 q half1, 64:96 k half0, 96:128 k half1
            nc.vector.tensor_copy(
                QaugT[0:32, 0].rearrange("p f h -> p (f h)"), qkst[0:32, :]
            )
            nc.vector.tensor_copy(
                KaugT[0:32, 0].rearrange("p f h -> p (f h)"), qkst[64:96, :]
            )
            nc.gpsimd.tensor_copy(
                QaugT[0:32, 1].rearrange("p f h -> p (f h)"), qkst[32:64, :]
            )
            nc.gpsimd.tensor_copy(
                KaugT[0:32, 1].rearrange("p f h -> p (f h)"), qkst[96:128, :]
            )

            out_sb = cp.tile([128, NT, NH * DH], FP)

            # rel views (interleaved): free ordering per mm is (hb, x|y)
            q_i = QaugT[0:32]                                  # [32,2,L,4]
            qr = q_i.rearrange("p a (x y) h -> p a h x y", y=W)
            wd = QaugT[32:64].rearrange("p a (x y) h -> p a h x y", y=W)
            hd = QaugT[64:96].rearrange("p a (x y) h -> p a h x y", y=W)

            def rel_group(pool, half, g, wdir, evac_engs, tag="rp"):
                # one y(or x)-group of 4 pre-skewed rel matmuls for one
                # 4-head half, then 2 evac halves
                rp = pool.tile([32, 4, 4, 32], FP, tag=tag, name=f"rp{half}_{wdir}_{g}")
                for i in range(4):
                    v = 4 * g + i
                    if wdir:
                        nc.tensor.matmul(
                            rp[:, i],
                            lhsT=krw_sb[:, 31 - v : 63 - v],
                            rhs=qr[:, half, :, :, v],
                            start=True,
                            stop=True,
                        )
                    else:
                        nc.tensor.matmul(
                            rp[:, i],
                            lhsT=krh_sb[:, 31 - v : 63 - v],
                            rhs=qr[:, half, :, v, :],
                            start=True,
                            stop=True,
                        )
                if wdir:
                    dst = wd[:, half, :, :, 4 * g : 4 * g + 4].rearrange(
                        "p h x i -> p i h x"
                    )
                else:
                    dst = hd[:, half, :, 4 * g : 4 * g + 4, :].rearrange(
                        "p h i y -> p i h y"
                    )
                copy_on(evac_engs[0], dst, rp)

            # ---------------- rel half 0 (heads 0-3) ----------------------
            rel0_rot = [(nc.vector, nc.scalar), (nc.scalar, nc.vector),
                        (nc.vector, nc.scalar), (nc.scalar, nc.vector)]
            with tc.tile_pool(name="ps_rel", bufs=6, space="PSUM") as ps_rel:
                for g in range(8):
                    rel_group(ps_rel, 0, g, True, rel0_rot[g % 4])
                for g in range(8):
                    rel_group(ps_rel, 0, g, False, rel0_rot[(g + 1) % 4])

            # ---------------- attention over heads ------------------------
            with (
                tc.tile_pool(name="wt", bufs=3) as wtp,
                tc.tile_pool(name="at", bufs=2) as atp,
                tc.tile_pool(name="sm", bufs=4) as smp,
                tc.tile_pool(name="ps_lt", bufs=2, space="PSUM") as ps_lt,
                tc.tile_pool(name="ps_av", bufs=2, space="PSUM") as ps_av,
            ):
                wts = {}
                avs = {}
                at_sbs = {}

                def evac_head(h, engs):
                    av = avs.pop(h)
                    at_sb = atp.tile([DH + 1, L], BF, tag="at", name=f"at{h}")
                    at_sbs[h] = at_sb
                    copy_on(engs[0], at_sb[:, 0:512], av[:, 0:512])
                    copy_on(engs[1], at_sb[:, 512:1024], av[:, 512:1024])

                def finish_ft(h, t0, t1, tt_eng):
                    at_sb = at_sbs[h]
                    ftile = ps_lt.tile(
                        [128, t1 - t0, DH + 2], BF, tag="lt", name=f"ft{h}_{t0}"
                    )
                    for t in range(t0, t1):
                        nc.tensor.transpose(
                            ftile[:, t - t0, 0 : DH + 1],
                            at_sb[:, t * 128 : (t + 1) * 128],
                            ident[0 : DH + 1, 0 : DH + 1],
                        )
                    rcp = smp.tile([128, t1 - t0], FP, tag="rcp")
                    nc.vector.reciprocal(rcp, ftile[:, :, DH])
                    rcp_b = bass.AP(
                        tensor=rcp.tensor,
                        offset=rcp.offset,
                        ap=[rcp.ap[0], rcp.ap[1], [0, DH]],
                    )
                    tt_eng.tensor_tensor(
                        out_sb[:, t0:t1, h * DH : (h + 1) * DH],
                        ftile[:, :, 0:DH],
                        rcp_b,
                        mybir.AluOpType.mult,
                    )

                # rel half 1 groups spread over phases 0-3 (heads 4-7 only
                # need them from phase 4); evacs on DVE (Pool cannot access
                # PSUM on real hardware)
                rel1 = [(g, True) for g in range(8)] + [(g, False) for g in range(8)]
                rel1_rot = [(nc.vector, nc.vector), (nc.vector, nc.vector)]

                for h in range(NH + 1):
                    if h < NH:
                        wts[h] = wtp.tile(
                            [128, NT, L], BF, tag="wt", name=f"wt{h}"
                        )
                        ha, hb = h // 4, h % 4
                    if h > 0:
                        avp = ps_av.tile([DH + 1, L], FP, tag="av", name=f"av{h-1}")
                        avs[h - 1] = avp
                        WTp = wts[h - 1]
                    for kt in range(NT):
                        if kt == 2 and h >= 2:
                            finish_ft(h - 2, 0, NT, nc.vector)
                            del at_sbs[h - 2]
                        if h < NH:
                            lt = ps_lt.tile([128, L], FP, tag="lt")
                            for qc in range(2):
                                nc.tensor.matmul(
                                    lt[:, qc * 512 : (qc + 1) * 512],
                                    lhsT=KaugT[:, ha, kt * 128 : (kt + 1) * 128, hb],
                                    rhs=QaugT[:, ha, qc * 512 : (qc + 1) * 512, hb],
                                    start=True,
                                    stop=True,
                                )
                            # exp may only run on the Activation engine
                            nc.scalar.activation(
                                wts[h][:, kt, :], lt, AF.Exp, scale=SCALE
                            )
                        if h < 4 and kt % 2 == 0:
                            g, wdir = rel1[4 * h + kt // 2]
                            rel_group(ps_av, 1, g, wdir, rel1_rot[kt % 2], tag="av")
                        if h > 0:
                            for qc in range(2):
                                nc.tensor.matmul(
                                    avp[:, qc * 512 : (qc + 1) * 512],
                                    lhsT=Vaug[:, kt, h - 1, 0 : DH + 1],
                                    rhs=WTp[:, kt, qc * 512 : (qc + 1) * 512],
                                    start=(kt == 0),
                                    stop=(kt == NT - 1),
                                )
                    if h > 0:
                        del wts[h - 1]
                        if h == NH:
                            evac_head(h - 1, (nc.vector, nc.scalar))
                        else:
                            evac_head(h - 1, (nc.vector, nc.vector))

                # tail: last head's finish interleaved with the out stores
                out_r = out.rearrange("(t p) c -> p t c", p=128)
                finish_ft(NH - 1, 0, NT // 2, nc.vector)
                for t in range(0, NT // 2, 2):
                    eng = (nc.sync, nc.scalar)[(t // 2) % 2]
                    eng.dma_start(
                        out=out_r[:, t : t + 2, :], in_=out_sb[:, t : t + 2, :]
                    )
                finish_ft(NH - 1, NT // 2, NT, nc.vector)
                for t in range(NT // 2, NT, 2):
                    eng = (nc.sync, nc.scalar)[(t // 2) % 2]
                    eng.dma_start(
                        out=out_r[:, t : t + 2, :], in_=out_sb[:, t : t + 2, :]
                    )
    nc.compile()
    return nc


_NC_CACHE = None


def kernel(inputs: np.ndarray, key_rel_w: np.ndarray, key_rel_h: np.ndarray) -> np.ndarray:
    global _NC_CACHE
    xf32 = inputs.astype(np.float32).reshape(B, L, 3 * NH * DH)
    xbf = np.ascontiguousarray(xf32.astype(ml_dtypes.bfloat16))
    # [g, hb, d, pos] -> [g*32+d, pos*4+hb]
    qki = np.ascontiguousarray(
        xf32[:, :, 0:512].transpose(0, 2, 1).reshape(B, 4, 4, 32, L)
        .transpose(0, 1, 3, 4, 2).reshape(B, 128, 4 * L)
        .astype(ml_dtypes.bfloat16)
    )
    krwhT = np.ascontiguousarray(
        np.concatenate([key_rel_w, key_rel_h], axis=0)
        .astype(np.float32).T.astype(ml_dtypes.bfloat16)
    )
    oneh = _build_onehot()

    if _NC_CACHE is None:
        _NC_CACHE = _build_nc()
    nc = _NC_CACHE

    identb = np.eye(128, dtype=np.float32).astype(ml_dtypes.bfloat16)
    in_maps = [
        {"xbf": xbf[b], "qki": qki[b], "krwhT": krwhT, "oneh": oneh,
         "identb": identb}
        for b in range(B)
    ]
    res = run_bass_kernel_spmd(nc, in_maps, list(range(B)))
    o = np.stack([res.results[b]["out"] for b in range(B)], axis=0)
    return np.ascontiguousarray(o.reshape(B, H, W, NH * DH).astype(np.float32))
